# revision 12
# baseline (speedup 1.0000x reference)
"""Causal self-attention (B=4, S=2048, E=1024, H=16, D=64) on 8 TRN2 cores.

Sharding: core c handles batch b = c//2 and heads [8*(c%2), 8*(c%2)+8).
Each core computes qkv for its 8 heads, full attention for them, and a
partial output projection; the host sums the two partial projections per
batch (the "all-reduce after proj" done host-side).

v3 structure:
  - The attention phase is ACT-bound (exp at 1 elem/cycle/lane @1.2GHz);
    the PE has slack inside each head-pair's window.  Only slab 0's QT/KT
    and the low half of V are computed up front; the rest of V, QT/KT for
    slab j+1 and the early output-projection row chunks are emitted as
    filler thunks popped once per attention k-chunk, which the Tile
    scheduler (priority heap over ready instructions) interleaves into
    the window at matmul granularity.
  - Scores tiles are per k-chunk with BOTH heads in one [128,2,512] psum
    tile: one exp covers both heads, the scores pipeline only needs 2 of
    the 3 psum slots, and the causal trim applies to scores, exp and AV
    alike (fully-masked leading columns of diagonal chunks are skipped).
  - The two heads' scores matmuls are row-disjoint (64x128) PE tiles that
    the array runs concurrently.
  - Normalization per (pair, qi): reciprocal_approx_fast on the
    denominator row straight out of the AV psum (row 64 = the ones column
    of V'), gpsimd partition-broadcast, one DVE mul into yt.
  - Inputs land as per-chunk tiles over 3 DMA queues in first-use order;
    the device emits bf16 partial projections (host sums in f32).
"""

import numpy as np
import ml_dtypes

import concourse.bass as bass
import concourse.bacc as bacc
import concourse.mybir as mybir
import concourse.tile as tile
from concourse.bass_utils import run_bass_kernel_spmd


BF16 = mybir.dt.bfloat16
F32 = mybir.dt.float32
AF = mybir.ActivationFunctionType

B, S, E = 4, 2048, 1024
H, D = 16, 64
HL = 8                # heads per core
DL = HL * D           # 512 local head dims
NSLAB = HL // 2       # 4 partition slabs of 2 heads (128 dims) each
KCH = E // 128        # 8 contraction chunks for the qkv matmuls
NQ = S // 512         # 4 query blocks of 512
NK = S // 128         # 16 key chunks of 128
NM = S // 128         # 16 output row chunks

_CACHE: dict = {}


def _emit(nc: bass.Bass, tc, ap):
    xt, wq, wk, wv, wp = ap["xt"], ap["wq"], ap["wk"], ap["wv"], ap["wp"]
    bq, bk, bv, bp, mk, out = ap["bq"], ap["bk"], ap["bv"], ap["bp"], ap["mk"], ap["out"]

    import contextlib
    ctx = contextlib.ExitStack()
    with ctx:
        const = ctx.enter_context(tc.tile_pool(name="const", bufs=1))
        exp_pool = ctx.enter_context(tc.tile_pool(name="exp", bufs=8))
        out_pool = ctx.enter_context(tc.tile_pool(name="outp", bufs=4))
        r_pool = ctx.enter_context(tc.tile_pool(name="rp", bufs=4))
        rb_pool = ctx.enter_context(tc.tile_pool(name="rbp", bufs=4))
        yb_pool = ctx.enter_context(tc.tile_pool(name="ybp", bufs=8))
        acc_ps = ctx.enter_context(tc.tile_pool(name="acc", bufs=2, space="PSUM"))
        sc_ps = ctx.enter_context(tc.tile_pool(name="scps", bufs=3, space="PSUM"))

        # ---- persistent SBUF tensors + input DMA ----
        # 3 DMA queues (sync / gpsimd / scalar), per-chunk tiles, ordered by
        # first use.  Scalar-queue triggers all land while ACT is idle.
        xt_sb = [const.tile([128, S], BF16, name=f"xt{k}") for k in range(KCH)]
        wq_sb = [const.tile([128, DL], BF16, name=f"wq{k}") for k in range(KCH)]
        wk_sb = [const.tile([128, DL], BF16, name=f"wk{k}") for k in range(KCH)]
        wv_sb = [const.tile([128, DL], BF16, name=f"wv{k}") for k in range(KCH)]
        wp_sb = [const.tile([128, E], BF16, name=f"wp{j}") for j in range(NSLAB)]
        for k in range(0, KCH, 2):
            nc.sync.dma_start(xt_sb[k][:, :], xt[k * 128:(k + 1) * 128, :])
        for k in range(KCH):
            nc.gpsimd.dma_start(wq_sb[k][:, :], wq[k * 128:(k + 1) * 128, :])
        for k in range(1, KCH, 2):
            nc.scalar.dma_start(xt_sb[k][:, :], xt[k * 128:(k + 1) * 128, :])
        bq_sb = const.tile([128, NSLAB], F32, name="bq_sb")
        nc.sync.dma_start(bq_sb[:, :], bq[:, :])
        bk_sb = const.tile([128, NSLAB], F32, name="bk_sb")
        nc.sync.dma_start(bk_sb[:, :], bk[:, :])
        for k in range(KCH):
            q = nc.gpsimd if k % 2 == 0 else nc.sync
            q.dma_start(wv_sb[k][:, :], wv[k * 128:(k + 1) * 128, :])
        for k in range(KCH):
            nc.scalar.dma_start(wk_sb[k][:, :], wk[k * 128:(k + 1) * 128, :])
        mk_sb = const.tile([128, 2, 128], BF16, name="mk_sb")
        nc.gpsimd.dma_start(mk_sb[:, :, :], mk[:, :])
        bv_sb = const.tile([128, DL], F32, name="bv_sb")
        nc.scalar.dma_start(bv_sb[:, :], bv[:, :])
        for j in range(NSLAB):
            nc.gpsimd.dma_start(wp_sb[j][:, :], wp[j * 128:(j + 1) * 128, :])
        bp_sb = const.tile([128, E], F32, name="bp_sb")
        nc.gpsimd.dma_start(bp_sb[:, :], bp[:, :])

        qt_sb = const.tile([128, NSLAB, S], BF16, name="qt_sb")
        kt_sb = const.tile([128, NSLAB, S], BF16, name="kt_sb")
        v_sb = const.tile([128, NK, HL, D + 1], BF16, name="v_sb")
        yt_sb = const.tile([128, NSLAB, S], BF16, name="yt_sb")

        # ---- qkv thunks ----
        # QT/KT for (slab j, sj-pair): 16 matmuls accumulating 2 query-column
        # blocks in one 2-bank psum tile (weights reload once per sj-pair).
        def qk_thunk(j, w_sb, b_sb, dst, sjp):
            def run():
                pa = sc_ps.tile([128, 2, 512], F32, name="pa", tag="sc")
                for k in range(KCH):
                    for s2 in range(2):
                        sj = 2 * sjp + s2
                        nc.tensor.matmul(
                            pa[:, s2, :],
                            lhsT=w_sb[k][:, j * 128:(j + 1) * 128],
                            rhs=xt_sb[k][:, sj * 512:(sj + 1) * 512],
                            start=(k == 0), stop=(k == KCH - 1),
                        )
                for s2 in range(2):
                    sj = 2 * sjp + s2
                    nc.vector.tensor_scalar_add(
                        dst[:, j, sj * 512:(sj + 1) * 512], pa[:, s2, :], b_sb[:, j:j + 1]
                    )
            return run

        # V for key-chunk pair (2*scp, 2*scp+1): all 8 heads at once (N=512),
        # with a ones column appended per head so the AV matmul also yields
        # the softmax denominator (row 64) for free.
        def v_thunk(scp):
            def run():
                ps = sc_ps.tile([128, 2, 512], F32, name="vps", tag="sc")
                for s2 in range(2):
                    sc = 2 * scp + s2
                    for k in range(KCH):
                        nc.tensor.matmul(
                            ps[:, s2, :],
                            lhsT=xt_sb[k][:, sc * 128:(sc + 1) * 128],
                            rhs=wv_sb[k][:, :],
                            start=(k == 0), stop=(k == KCH - 1),
                        )
                for s2 in range(2):
                    sc = 2 * scp + s2
                    nc.vector.tensor_add(
                        v_sb[:, sc, :, 0:D],
                        ps[:, s2, :].rearrange("p (h d) -> p h d", h=HL),
                        bv_sb.rearrange("p (h d) -> p h d", h=HL),
                    )
                    nc.vector.memset(v_sb[:, sc, :, D:D + 1], 1.0)
            return run

        # Output projection for row chunk m: partial over all 4 slabs; both
        # 512-wide halves accumulate in one 2-bank psum tile sharing the
        # stationary yt chunk.  Emits bf16 partials (host sums in f32).
        def proj_thunk(m):
            def run():
                pp = sc_ps.tile([128, 2, 512], F32, name="pp", tag="sc")
                for j in range(NSLAB):
                    for n in range(2):
                        nc.tensor.matmul(
                            pp[:, n, :],
                            lhsT=yt_sb[:, j, m * 128:(m + 1) * 128],
                            rhs=wp_sb[j][:, n * 512:(n + 1) * 512],
                            start=(j == 0), stop=(j == NSLAB - 1),
                        )
                for n in range(2):
                    o_t = out_pool.tile([128, 512], BF16, name="o_t", tag="ot")
                    nc.vector.tensor_add(o_t[:, :], pp[:, n, :], bp_sb[:, n * 512:(n + 1) * 512])
                    nc.sync.dma_start(out[m * 128:(m + 1) * 128, n * 512:(n + 1) * 512], o_t[:, :])
            return run

        # ---- attention for head pair j ----
        # One [128,2,512] scores tile per k-chunk (both heads), one exp per
        # chunk.
        def attn_pair(j):
            scale = float(D) ** -0.5
            for qi in range(NQ):
                nk = 4 * (qi + 1)  # causal: k chunks 0..nk-1 needed
                us = [acc_ps.tile([128, 512], F32, name="u_ps", tag="acc")
                      for _ in range(2)]
                for kc in range(nk):
                    dc = kc - 4 * qi
                    tr = 128 * dc if 0 <= dc <= 3 else 0
                    sct = sc_ps.tile([128, 2, 512], F32, name="sc_t", tag="sc")
                    ext = exp_pool.tile([128, 2, 512], BF16, name="ex_t", tag="ex")
                    for i in range(2):
                        nc.tensor.matmul(
                            sct[:, i, tr:512],
                            lhsT=kt_sb[64 * i:64 * i + 64, j, kc * 128:(kc + 1) * 128],
                            rhs=qt_sb[64 * i:64 * i + 64, j, qi * 512 + tr:(qi + 1) * 512],
                            start=True, stop=True,
                        )
                    nc.scalar.activation(
                        ext[:, :, tr:512], sct[:, :, tr:512], AF.Exp, scale=scale,
                    )
                    if 0 <= dc <= 3:
                        # both heads' 128-wide triangle blocks in one mul
                        nc.vector.tensor_mul(
                            ext[:, :, tr:tr + 128], ext[:, :, tr:tr + 128], mk_sb[:, :, :]
                        )
                    for i in range(2):
                        nc.tensor.matmul(
                            us[i][0:D + 1, tr:512],
                            lhsT=v_sb[:, kc, 2 * j + i, :],
                            rhs=ext[:, i, tr:512],
                            start=(kc == 0), stop=(kc == nk - 1),
                        )
                # normalization for (pair, qi): approx reciprocal of the
                # denominator row straight out of psum, gpsimd broadcast,
                # one DVE mul into yt.  yb copy releases the psum bank.
                for i in range(2):
                    rr0_t = r_pool.tile([1, 512], F32, name="rr0_t", tag="rr")
                    nc.vector.tensor_copy(rr0_t[:, :], us[i][D:D + 1, :])
                    rr_t = r_pool.tile([1, 512], F32, name="rr_t", tag="rr")
                    nc.vector.reciprocal_approx_fast(rr_t[:, :], rr0_t[:, :])
                    yb_t = yb_pool.tile([64, 512], BF16, name="yb_t", tag="yb")
                    nc.vector.tensor_copy(yb_t[:, :], us[i][0:D, :])
                    rb_t = rb_pool.tile([64, 512], F32, name="rb_t", tag="rb")
                    nc.gpsimd.partition_broadcast(rb_t[:, :], rr_t[:, :])
                    nc.vector.tensor_mul(
                        yt_sb[64 * i:64 * i + 64, j, qi * 512:(qi + 1) * 512],
                        yb_t[:, :], rb_t[:, :],
                    )

        # ---- emission schedule ----
        # Natural priority: just what pair 0's first chunks need (QT/KT slab
        # 0 low query half, V key chunks 0-7), the attention pairs, and the
        # projection.  Everything else is emitted up front (so dependency
        # tracking sees writes before reads) but at background priority:
        # the scheduler runs it only when the PE would otherwise idle, so
        # the ACT-bound attention pipeline is never displaced.
        qk_thunk(0, wq_sb, bq_sb, qt_sb, 0)()
        qk_thunk(0, wk_sb, bk_sb, kt_sb, 0)()
        v_thunk(0)()
        v_thunk(1)()
        with tc.high_priority(offset=-400000):
            qk_thunk(0, wq_sb, bq_sb, qt_sb, 1)()
            qk_thunk(0, wk_sb, bk_sb, kt_sb, 1)()
            for scp in range(2, 8):
                v_thunk(scp)()
            for j in range(1, NSLAB):
                for sjp in range(2):
                    qk_thunk(j, wq_sb, bq_sb, qt_sb, sjp)()
                    qk_thunk(j, wk_sb, bk_sb, kt_sb, sjp)()

        for j in range(NSLAB):
            attn_pair(j)

        for m in range(NM):
            proj_thunk(m)()


def build():
    if "nc" in _CACHE:
        return _CACHE["nc"]
    nc = bacc.Bacc("TRN2", debug=False)
    ap = {
        "xt": nc.dram_tensor("xt", [E, S], BF16, kind="ExternalInput").ap(),
        "wq": nc.dram_tensor("wq", [E, DL], BF16, kind="ExternalInput").ap(),
        "wk": nc.dram_tensor("wk", [E, DL], BF16, kind="ExternalInput").ap(),
        "wv": nc.dram_tensor("wv", [E, DL], BF16, kind="ExternalInput").ap(),
        "wp": nc.dram_tensor("wp", [DL, E], BF16, kind="ExternalInput").ap(),
        "bq": nc.dram_tensor("bq", [128, NSLAB], F32, kind="ExternalInput").ap(),
        "bk": nc.dram_tensor("bk", [128, NSLAB], F32, kind="ExternalInput").ap(),
        "bv": nc.dram_tensor("bv", [128, DL], F32, kind="ExternalInput").ap(),
        "bp": nc.dram_tensor("bp", [128, E], F32, kind="ExternalInput").ap(),
        "mk": nc.dram_tensor("mk", [128, 256], BF16, kind="ExternalInput").ap(),
        "out": nc.dram_tensor("out", [S, E], BF16, kind="ExternalOutput").ap(),
    }
    with tile.TileContext(nc) as tc:
        _emit(nc, tc, ap)
    nc.compile()
    _CACHE["nc"] = nc
    return nc


def make_in_maps(x, w_qkv, b_qkv, w_proj, b_proj):
    """Host-side sharding: one input map per core."""
    bf = ml_dtypes.bfloat16
    in_maps = []
    for c in range(8):
        b, half = c // 2, c % 2
        hbase = half * HL
        dsl = slice(hbase * D, hbase * D + DL)
        xt = np.ascontiguousarray(x[b].T).astype(bf)
        wqs = np.ascontiguousarray(w_qkv[:, 0 * E:1 * E][:, dsl]).astype(bf)
        wks = np.ascontiguousarray(w_qkv[:, 1 * E:2 * E][:, dsl]).astype(bf)
        wvs = np.ascontiguousarray(w_qkv[:, 2 * E:3 * E][:, dsl]).astype(bf)
        wps = np.ascontiguousarray(w_proj[dsl, :]).astype(bf)
        bqs = np.ascontiguousarray(
            b_qkv[0 * E:1 * E][dsl].reshape(NSLAB, 128).T).astype(np.float32)
        bks = np.ascontiguousarray(
            b_qkv[1 * E:2 * E][dsl].reshape(NSLAB, 128).T).astype(np.float32)
        bvs = np.broadcast_to(b_qkv[2 * E:3 * E][dsl], (128, DL)).astype(np.float32)
        # both cores of a batch pair add bp and the host sums them: halve it
        bps = np.broadcast_to(b_proj * 0.5, (128, E)).astype(np.float32)
        kk = np.arange(128)[:, None]
        qq = np.arange(128)[None, :]
        tri = (kk <= qq).astype(bf)
        mks = np.concatenate([tri, tri], axis=1)  # [128, 256]: one per head
        in_maps.append({
            "xt": xt, "wq": wqs, "wk": wks, "wv": wvs, "wp": wps,
            "bq": bqs, "bk": bks, "bv": np.ascontiguousarray(bvs),
            "bp": np.ascontiguousarray(bps), "mk": np.ascontiguousarray(mks),
        })
    return in_maps


def kernel(x, w_qkv, b_qkv, w_proj, b_proj, _trace=False):
    x = np.asarray(x, np.float32)
    w_qkv = np.asarray(w_qkv, np.float32)
    b_qkv = np.asarray(b_qkv, np.float32)
    w_proj = np.asarray(w_proj, np.float32)
    b_proj = np.asarray(b_proj, np.float32)
    nc = build()
    in_maps = make_in_maps(x, w_qkv, b_qkv, w_proj, b_proj)
    res = run_bass_kernel_spmd(nc, in_maps, core_ids=list(range(8)), trace=_trace)
    _CACHE["last_results"] = res
    out = np.empty((B, S, E), dtype=np.float32)
    for b in range(B):
        out[b] = (res.results[2 * b]["out"].astype(np.float32)
                  + res.results[2 * b + 1]["out"].astype(np.float32))
    return out
